# revision 36
# baseline (speedup 1.0000x reference)
"""Trainium2 Bass kernel for nn_EntityLinker (ragged_sequence).

Pure data-parallel over batch: 1024 batches -> 8 cores x 128 batches.

Gather strategy: the SWDGE fixed overhead (994ns/call) makes per-pair
indirect DMACopy gathers (576 calls/core) the bottleneck, so instead we use
the custom GPSIMD dma_gather instruction (994ns + 0.34ns/row per call,
<=896 rows/call from the descriptor-ring cap).  dma_gather needs int16
indices, so the host dedups each half-core's referenced embedding rows
(~31K distinct < 32768) into a per-half fp16 table and remaps indices to
table-local int16.  The device still gathers every reference (36MB/core)
through the DMA engines; the host only does O(refs) integer prep plus a
table layout transform.

Token sums over T=8 column tokens are fp16 identity-matmuls accumulating
in PSUM.  Attention + MLP run 2 batches per 128-partition tile, fp16
operands with fp32 PSUM accumulation, elementwise ops merged over duos
(2 pairs) to halve fixed per-op latency.  The whole per-duo dependency
chain is software-pipelined ACROSS group boundaries (engines execute
their streams in order, so a per-group pipeline would drain at each
boundary and fall behind the gather stream).  b_o is added on host.
"""

import sys

if "/opt/trn_rl_repo" not in sys.path:
    sys.path.insert(0, "/opt/trn_rl_repo")

import numpy as np

V, D = 100000, 128
B, Q, C, T = 1024, 64, 64, 8
NCORES = 8
BL = B // NCORES        # 128 batches per core
PAIRS = BL // 2         # 64 pairs (2 batches per 128-partition tile)
GP = 16                 # pairs per group
NG = PAIRS // GP        # 4 groups
ND = GP // 2            # duos per group
NDG = ND * NG           # duos per core
NB = GP + GP * T        # gather blocks per group: q then c
NH = 2                  # table halves per core (2 groups each)
TBL = 32768             # table rows per half (int16-addressable)
NEG = np.float32(-20000.0)   # fp16-safe mask bias
SCALE_SIM = float(1.0 / np.sqrt(128.0))

_P_H = np.arange(128) // 64     # which batch of the pair this partition holds
_P_C = np.arange(128) % 64      # column / q index within the batch


def _wrap16(flat):
    """dma_gather index layout: element i -> partition i%16, col i//16,
    replicated across the 8 GPSIMD sub-cores (128 partitions)."""
    w = flat.reshape(-1, 16).T          # [16, n//16]
    return np.ascontiguousarray(np.tile(w, (8, 1)))  # [128, n//16]


def _prep_core(core, q_ids, c_ids, num_qs, cnt, embed16):
    base = core * BL
    jj = np.arange(GP)

    out = dict()
    for h in range(NH):
        hb = base + h * (BL // NH)            # first batch of the half
        qh = q_ids[hb:hb + BL // NH]          # [64, Q]
        ch = c_ids[hb:hb + BL // NH]          # [64, C, T]
        refs = np.concatenate([qh.ravel(), ch.ravel()])
        uniq, inv = np.unique(refs, return_inverse=True)
        assert len(uniq) <= TBL, f"half table overflow: {len(uniq)}"
        tab = np.zeros((TBL, D), np.float16)
        tab[:len(uniq)] = embed16[uniq]
        out[f"tab{h}"] = tab
        nq = Q * (BL // NH)
        out[f"inv_q{h}"] = inv[:nq].reshape(BL // NH, Q).astype(np.int16)
        out[f"inv_c{h}"] = inv[nq:].reshape(BL // NH, C, T).astype(np.int16)

    q_idx = np.empty((NG * 128, (GP * 128) // 16), np.int16)
    c_idx = np.empty((NG * 128, (GP * T * 128) // 16), np.int16)
    qbias = np.empty((NG * 2, GP * 128), np.float16)
    qv = np.zeros((NG * 128, 2 * GP), np.float16)
    cscale = np.empty((NG * 128, GP), np.float32)

    for g in range(NG):
        h = g // 2
        inv_q = out[f"inv_q{h}"]
        inv_c = out[f"inv_c{h}"]
        # local (within-half) batch index per (partition, pair)
        lb = (g % 2) * 2 * GP + 2 * jj[None, :] + _P_H[:, None]   # [128, GP]
        cm = np.broadcast_to(_P_C[:, None], lb.shape)             # [128, GP]

        # q slots first: i = j*128 + pc
        qi = inv_q[lb, cm]                                        # [128, GP]
        # then c slots duo-major: i = u*2048 + t*256 + p2*128 + pc, so the
        # two pairs of a duo land in adjacent blocks for merged T-sums
        ci = inv_c[lb, cm]                                        # [128, GP, T]
        ci_r = ci.transpose(1, 2, 0).reshape(GP // 2, 2, T, 128)  # u, p2, t, pc
        q_idx[g * 128:(g + 1) * 128] = _wrap16(qi.T.ravel())
        c_idx[g * 128:(g + 1) * 128] = _wrap16(
            ci_r.transpose(0, 2, 1, 3).ravel())

        gbat = base + h * (BL // NH) + lb                         # global batch
        cscale[g * 128:(g + 1) * 128] = 1.0 / cnt[gbat, cm]
        for r in range(2):
            bvec = base + g * 2 * GP + 2 * jj + r
            nqs = num_qs[bvec]
            blk = np.full((GP, 128), NEG, np.float32)
            blk[:, r * 64:(r + 1) * 64] = np.where(
                np.arange(64)[None, :] < nqs[:, None], np.float32(0.0), NEG)
            qbias[g * 2 + r] = blk.reshape(-1).astype(np.float16)
            valid = (_P_C[:, None] < nqs[None, :]) & (_P_H[:, None] == r)
            qv[g * 128:(g + 1) * 128, 2 * jj + r] = \
                (valid / nqs[None, :]).astype(np.float16)

    return dict(tab0=out["tab0"], tab1=out["tab1"], q_idx=q_idx,
                c_idx=c_idx, qbias=qbias, qv=qv, cscale=cscale)


def prep_all(q_ids, c_ids, num_qs, embed):
    q_ids = np.asarray(q_ids).astype(np.int32)
    c_ids = np.asarray(c_ids).astype(np.int32)
    num_qs = np.asarray(num_qs).astype(np.int64)
    cnt = np.maximum((c_ids != 0).sum(-1), 1).astype(np.float32)     # [B, C]
    embed16 = np.asarray(embed, np.float32).astype(np.float16)
    return [_prep_core(i, q_ids, c_ids, num_qs, cnt, embed16)
            for i in range(NCORES)]


_BLOCKIND = np.zeros((2, 128), np.float16)
_BLOCKIND[0, :64] = 1.0
_BLOCKIND[1, 64:] = 1.0


def _build_program():
    from contextlib import ExitStack

    import concourse.bass as bass
    from concourse import bacc, mybir, tile
    from concourse.library_config import mlp
    from concourse.masks import make_identity

    f32 = mybir.dt.float32
    f16 = mybir.dt.float16
    i16 = mybir.dt.int16

    nc = bacc.Bacc("TRN2", target_bir_lowering=False, debug=False,
                   enable_asserts=False, num_devices=NCORES)

    tab0_d = nc.dram_tensor("tab0", [TBL, D], f16, kind="ExternalInput").ap()
    tab1_d = nc.dram_tensor("tab1", [TBL, D], f16, kind="ExternalInput").ap()
    whk_d = nc.dram_tensor("whk", [128, 5 * 128], f16, kind="ExternalInput").ap()
    w_o_d = nc.dram_tensor("w_o", [D, 1], f16, kind="ExternalInput").ap()
    b_h_d = nc.dram_tensor("b_h", [D, 1], f32, kind="ExternalInput").ap()
    blockind_d = nc.dram_tensor("blockind", [2, 128], f16, kind="ExternalInput").ap()
    q_idx_d = nc.dram_tensor("q_idx", [NG * 128, (GP * 128) // 16], i16,
                             kind="ExternalInput").ap()
    c_idx_d = nc.dram_tensor("c_idx", [NG * 128, (GP * T * 128) // 16], i16,
                             kind="ExternalInput").ap()
    qbias_d = nc.dram_tensor("qbias", [NG * 2, GP * 128], f16, kind="ExternalInput").ap()
    qv_d = nc.dram_tensor("qv", [NG * 128, 2 * GP], f16, kind="ExternalInput").ap()
    cscale_d = nc.dram_tensor("cscale", [NG * 128, GP], f32, kind="ExternalInput").ap()
    out_d = nc.dram_tensor("out", [128, PAIRS], f32, kind="ExternalOutput").ap()

    Alu = mybir.AluOpType

    with tile.TileContext(nc) as tc, ExitStack() as ctx:
        const = ctx.enter_context(tc.tile_pool(name="const", bufs=1))
        gpool = ctx.enter_context(tc.tile_pool(name="gather", bufs=3))
        spool = ctx.enter_context(tc.tile_pool(name="work", bufs=5))
        ppool = ctx.enter_context(tc.tile_pool(name="psum", bufs=5, space="PSUM"))
        tpool = ctx.enter_context(tc.tile_pool(name="psumt", bufs=2, space="PSUM"))
        gpsum = ctx.enter_context(tc.tile_pool(name="gps", bufs=1, space="PSUM"))
        hpool = ctx.enter_context(tc.tile_pool(name="hbuf", bufs=ND + 3))

        ident = const.tile([128, 128], f16)
        make_identity(nc, ident[:])
        nc.gpsimd.load_library(mlp)

        whk = const.tile([128, 5 * 128], f16)
        nc.sync.dma_start(whk[:], whk_d[:])
        w_o_t = const.tile([128, 1], f16)
        nc.sync.dma_start(w_o_t[:], w_o_d[:])
        b_h_t = const.tile([128, 1], f32)
        nc.sync.dma_start(b_h_t[:], b_h_d[:])
        blockind_t = const.tile([2, 128], f16)
        nc.sync.dma_start(blockind_t[:], blockind_d[:])
        out_sb = const.tile([128, PAIRS], f32)

        Act = mybir.ActivationFunctionType

        groups = [dict() for _ in range(NG)]
        st = [dict() for _ in range(NDG)]

        def tab_of(g):
            return tab0_d if g < NG // 2 else tab1_d

        def emit_masks(g):
            """HWDGE mask/index loads: cheap, run for all groups up front."""
            G = groups[g]
            G["qidx"] = gpool.tile([128, (GP * 128) // 16], i16, tag="qidx",
                                   name="qidx", bufs=NG)
            G["qbias"] = gpool.tile([2, GP * 128], f16, tag="qbias",
                                    name="qbias", bufs=NG)
            G["qv"] = gpool.tile([128, 2 * GP], f16, tag="qv", name="qv",
                                 bufs=NG)
            G["csc"] = gpool.tile([128, GP], f32, tag="csc", name="csc",
                                  bufs=NG)
            nc.sync.dma_start(G["qidx"][:], q_idx_d[g * 128:(g + 1) * 128, :])
            nc.sync.dma_start(G["qbias"][:], qbias_d[g * 2:g * 2 + 2, :])
            nc.sync.dma_start(G["qv"][:], qv_d[g * 128:(g + 1) * 128, :])
            nc.sync.dma_start(G["csc"][:], cscale_d[g * 128:(g + 1) * 128, :])

        def emit_q_io(g):
            """q gathers for group g (issued >=1 group ahead of its compute
            so the prologue's qs matmuls never stall the in-order engine
            streams waiting on fresh gather data)."""
            G = groups[g]
            qdst = gpool.tile([128, GP, 128], f16, tag="qdst", name="qdst",
                              bufs=NG)
            G["qdst"] = qdst
            s = 0
            while s < GP:
                m = min(7, GP - s)
                nc.gpsimd.dma_gather(qdst[:, s:s + m, :], tab_of(g)[:],
                                     G["qidx"][:, s * 8:(s + m) * 8],
                                     m * 128, m * 128, D)
                s += m

        def emit_group_io(g, upto=GP * T):
            """Chunked dma_gather of the group's c tokens (duo-major).
            ``upto`` allows splitting the chunk stream so startup q gathers
            can interleave after the first few c chunks."""
            G = groups[g]
            if "cidx" not in G:
                G["cidx"] = gpool.tile([128, (GP * T * 128) // 16], i16,
                                       tag="cidx", name="cidx")
                nc.sync.dma_start(G["cidx"][:],
                                  c_idx_d[g * 128:(g + 1) * 128, :])
                G["dest"] = gpool.tile([128, GP * T, 128], f16, tag="dst",
                                       name="dest")
                G["c_done"] = 0
            dest, cidx_t = G["dest"], G["cidx"]
            s = G["c_done"]
            while s < upto:
                m = min(7, GP * T - s)
                nc.gpsimd.dma_gather(dest[:, s:s + m, :], tab_of(g)[:],
                                     cidx_t[:, s * 8:(s + m) * 8],
                                     m * 128, m * 128, D)
                s += m
            G["c_done"] = s

        # Prologue (q_summary -> per-batch MLP bias columns, transposed to
        # biasT[r, j*128+d]) is split into three stages emitted at the END
        # of successive iterations so its data wait on the fresh q gather
        # never head-of-line-blocks the per-duo pipeline.
        def prologue_1(g):
            G = groups[g]
            qdst = G["qdst"]
            qs_psA = ppool.tile([128, 512], f32, tag="psA", name="qs_psA")
            qs_ps = qs_psA[:, 0:2 * GP]
            for j in range(GP):
                nc.tensor.matmul(qs_ps[:, 2 * j:2 * j + 2],
                                 lhsT=qdst[:, j, :],
                                 rhs=G["qv"][:, 2 * j:2 * j + 2],
                                 start=True, stop=True)
            G["qs_sb"] = spool.tile([128, 2 * GP], f16, tag="qs_sb",
                                    name="qs_sb")
            nc.vector.tensor_copy(G["qs_sb"][:], qs_ps)

        def prologue_2(g):
            G = groups[g]
            bias_psA = ppool.tile([128, 512], f32, tag="psA", name="bias_psA")
            bias_ps = bias_psA[:, 0:2 * GP]
            nc.tensor.matmul(bias_ps, lhsT=whk[:, 0:128], rhs=G["qs_sb"][:],
                             start=True, stop=True)
            G["bias_sb"] = spool.tile([128, 2 * GP], f16, tag="bias_sb",
                                      name="bias_sb")
            nc.scalar.activation(G["bias_sb"][:], bias_ps, Act.Identity,
                                 bias=b_h_t[:, 0:1])

        def prologue_3(g):
            G = groups[g]
            bias_sb = G["bias_sb"]
            biasT = spool.tile([2, GP * 128], f16, tag="biasT", name="biasT")
            G["biasT"] = biasT
            for quarter in range(4):
                bt_ps = tpool.tile([128, 768], f16, tag="pst", name="bt_ps")
                for jj_ in range(4):
                    j = quarter * 4 + jj_
                    nc.tensor.transpose(bt_ps[0:2, jj_ * 128:(jj_ + 1) * 128],
                                        bias_sb[:, 2 * j:2 * j + 2], ident[:])
                nc.vector.tensor_copy(
                    biasT[:, quarter * 512:(quarter + 1) * 512],
                    bt_ps[0:2, 0:512])

        def phase_a(u):
            g, ul = u // ND, u % ND
            G, s = groups[g], st[u]
            s["psA1"] = ppool.tile([128, 512], f32, tag="psA", name="psA1")
            cs = s["psA1"][:, 0:256]
            for t in range(T):
                b = ul * 2 * T + t * 2
                nc.tensor.matmul(cs, lhsT=ident[:],
                                 rhs=G["dest"][:, b:b + 2, :],
                                 start=(t == 0), stop=(t == T - 1))
            s["c_h2"] = spool.tile([128, 256], f16, tag="c_h2", name="c_h2")
            for p in range(2):
                nc.vector.tensor_scalar_mul(
                    s["c_h2"][:, p * 128:(p + 1) * 128],
                    s["psA1"][:, p * 128:(p + 1) * 128],
                    G["csc"][:, 2 * ul + p:2 * ul + p + 1])

        def phase_b_pre(u):
            g, ul = u // ND, u % ND
            G, s = groups[g], st[u]
            s["psT"] = tpool.tile([128, 768], f16, tag="pst", name="psT")
            for p in range(2):
                nc.tensor.transpose(s["psT"][:, p * 256:p * 256 + 128],
                                    s["c_h2"][:, p * 128:(p + 1) * 128],
                                    ident[:])
                nc.tensor.transpose(s["psT"][:, p * 256 + 128:p * 256 + 256],
                                    G["qdst"][:, 2 * ul + p, :], ident[:])

        def phase_b_post(u):
            g, ul = u // ND, u % ND
            G, s = groups[g], st[u]
            s["cq_hT"] = spool.tile([128, 512], f16, tag="cq_hT", name="cq_hT")
            nc.scalar.copy(s["cq_hT"][:], s["psT"][:, 0:512])
            for p in range(2):
                j = 2 * ul + p
                sim = s["psA1"][:, 256 + p * 128:256 + (p + 1) * 128]
                nc.tensor.matmul(sim, lhsT=s["cq_hT"][:, p * 256:p * 256 + 128],
                                 rhs=s["cq_hT"][:, p * 256 + 128:p * 256 + 256],
                                 start=True, stop=False)
                nc.tensor.matmul(sim, lhsT=blockind_t[:],
                                 rhs=G["qbias"][:, j * 128:(j + 1) * 128],
                                 start=False, stop=True)
            s["att_e"] = spool.tile([128, 256], f16, tag="att_e", name="att_e")
            nc.scalar.activation(s["att_e"][:], s["psA1"][:, 256:512],
                                 Act.Exp, scale=SCALE_SIM)

        def phase_c1(u):
            g, ul = u // ND, u % ND
            G, s = groups[g], st[u]
            s_cols = spool.tile([128, 2], f32, tag="s_cols", name="s_cols")
            nc.vector.tensor_reduce(
                s_cols[:],
                s["att_e"][:].rearrange("p (x d) -> p x d", d=128),
                axis=mybir.AxisListType.X, op=Alu.add)
            r_cols = spool.tile([128, 2], f32, tag="r_cols", name="r_cols")
            nc.vector.reciprocal(r_cols[:], s_cols[:])
            att = spool.tile([128, 256], f16, tag="att", name="att")
            for p in range(2):
                nc.vector.tensor_scalar_mul(
                    att[:, p * 128:(p + 1) * 128],
                    s["att_e"][:, p * 128:(p + 1) * 128],
                    r_cols[:, p:p + 1])
                nc.tensor.transpose(
                    s["psT"][:, 512 + p * 128:512 + (p + 1) * 128],
                    att[:, p * 128:(p + 1) * 128], ident[:])
            s["attT"] = spool.tile([128, 256], f16, tag="attT", name="attT")
            nc.scalar.copy(s["attT"][:], s["psT"][:, 512:768])

        def phase_c2(u):
            g, ul = u // ND, u % ND
            G, s = groups[g], st[u]
            for p in range(2):
                nc.tensor.matmul(s["psA1"][:, p * 128:(p + 1) * 128],
                                 lhsT=G["qdst"][:, 2 * ul + p, :],
                                 rhs=s["attT"][:, p * 128:(p + 1) * 128],
                                 start=True, stop=True)
            wqT = spool.tile([128, 256], f16, tag="wqT", name="wqT")
            s["wqT"] = wqT
            nc.vector.tensor_copy(wqT[:], s["psA1"][:, 0:256])
            ch3 = spool.tile([128, 256], f16, tag="ch3", name="ch3")
            dif = spool.tile([128, 256], f16, tag="dif", name="dif")
            for p in range(2):
                c_hT = s["cq_hT"][:, p * 256:p * 256 + 128]
                wqT_p = wqT[:, p * 128:(p + 1) * 128]
                nc.vector.tensor_mul(ch3[:, p * 128:(p + 1) * 128], c_hT, wqT_p)
                nc.vector.tensor_sub(dif[:, p * 128:(p + 1) * 128], c_hT, wqT_p)
            s["ch3"] = ch3
            ch4 = spool.tile([128, 256], f16, tag="ch4", name="ch4")
            nc.scalar.activation(ch4[:], dif[:], Act.Abs)
            s["ch4"] = ch4

        def phase_d(u):
            g, ul = u // ND, u % ND
            G, s = groups[g], st[u]
            for p in range(2):
                j = 2 * ul + p
                h = s["psA1"][:, 256 + p * 128:256 + (p + 1) * 128]
                c_hT = s["cq_hT"][:, p * 256:p * 256 + 128]
                sl = slice(p * 128, (p + 1) * 128)
                for k, rhs in ((1, c_hT), (2, s["wqT"][:, sl]),
                               (3, s["ch3"][:, sl]), (4, s["ch4"][:, sl])):
                    nc.tensor.matmul(h, lhsT=whk[:, k * 128:(k + 1) * 128],
                                     rhs=rhs, start=(k == 1), stop=False)
                nc.tensor.matmul(h, lhsT=G["biasT"][:, j * 128:(j + 1) * 128],
                                 rhs=blockind_t[:], start=False, stop=True)
            s["hT"] = hpool.tile([128, 256], f16, tag="hT", name="hT")
            nc.scalar.activation(s["hT"][:], s["psA1"][:, 256:512], Act.Tanh)

        def emit_epilogue(g):
            out_ps = gpsum.tile([128, GP], f32, tag="outp", name="out_ps")
            for ul in range(ND):
                s = st[g * ND + ul]
                for p in range(2):
                    nc.tensor.matmul(
                        out_ps[:, 2 * ul + p:2 * ul + p + 1],
                        lhsT=s["hT"][:, p * 128:(p + 1) * 128],
                        rhs=w_o_t[:], start=True, stop=True)
            nc.scalar.activation(out_sb[:, g * GP:(g + 1) * GP], out_ps[:],
                                 Act.Identity)

        # Global software pipeline: phases staggered across ALL duos so no
        # engine stream drains at group boundaries.  Within an iteration the
        # OLDEST phases are emitted first: engines execute their streams in
        # order, so putting the newest (gather-data-dependent) ops first
        # would head-of-line-block ready work for older duos.
        # Startup: masks first (HWDGE), a few c chunks so compute starts
        # immediately, then the q gathers (q for group g is always issued
        # >=1 group ahead of its compute).
        for g in range(NG):
            emit_masks(g)
        emit_group_io(0, upto=21)
        emit_q_io(0)
        emit_q_io(1)
        for i in range(NDG + 6):
            if i < NDG and i % ND == 0:
                g = i // ND
                emit_group_io(g)
                if g + 2 < NG:
                    emit_q_io(g + 2)
            if 5 <= i and i - 5 < NDG:
                phase_d(i - 5)
            if 4 <= i and i - 4 < NDG:
                phase_c2(i - 4)
            if 3 <= i and i - 3 < NDG:
                phase_c1(i - 3)
            if 2 <= i and i - 2 < NDG:
                phase_b_post(i - 2)
            if 1 <= i and i - 1 < NDG:
                phase_b_pre(i - 1)
            if i < NDG:
                phase_a(i)
            if 1 <= i and (i - 1) % ND == 0 and (i - 1) // ND < NG:
                prologue_1((i - 1) // ND)
            if 2 <= i and (i - 2) % ND == 0 and (i - 2) // ND < NG:
                prologue_2((i - 2) // ND)
            if 3 <= i and (i - 3) % ND == 0 and (i - 3) // ND < NG:
                prologue_3((i - 3) // ND)
            if i >= ND + 4 and (i - ND - 4) % ND == 0 and (i - ND - 4) // ND < NG:
                emit_epilogue((i - ND - 4) // ND)

        nc.sync.dma_start(out_d[:], out_sb[:])

    nc.compile()
    return nc


_PROGRAM = None


def _get_program():
    global _PROGRAM
    if _PROGRAM is None:
        _PROGRAM = _build_program()
    return _PROGRAM


def run_on_hw(in_maps, trace=False, **kw):
    from concourse import bass_utils
    nc = _get_program()
    return bass_utils.run_bass_kernel_spmd(
        nc, in_maps, core_ids=list(range(NCORES)), trace=trace, **kw)


def make_in_maps(q_ids, c_ids, num_qs, num_cols, embed, W_h, b_h, W_o, b_o):
    W_h = np.asarray(W_h, np.float32)
    whk = np.ascontiguousarray(
        W_h.reshape(5, 128, 128).transpose(1, 0, 2).reshape(128, 5 * 128)
    ).astype(np.float16)
    w_o = np.ascontiguousarray(
        np.asarray(W_o, np.float32).reshape(D, 1)).astype(np.float16)
    b_h_col = np.ascontiguousarray(
        np.asarray(b_h, np.float32).reshape(D, 1))
    shared = dict(whk=whk, w_o=w_o, b_h=b_h_col, blockind=_BLOCKIND)
    percore = prep_all(q_ids, c_ids, num_qs, embed)
    return [dict(shared, **percore[i]) for i in range(NCORES)]


def gather_out(res, b_o):
    b_o_val = np.float32(np.asarray(b_o).reshape(-1)[0])
    outs = np.empty((B, C, 1), np.float32)
    for i in range(NCORES):
        o = np.asarray(res.results[i]["out"], np.float32)  # [pc, j]
        # pc = 64*r + col ; batch = i*BL + 2*j + r
        o = o.reshape(2, 64, PAIRS)          # [r, col, j]
        o = o.transpose(2, 0, 1).reshape(BL, C)   # [(j, r), col]
        outs[i * BL:(i + 1) * BL, :, 0] = o + b_o_val
    return outs


def kernel(q_ids, c_ids, num_qs, num_cols, embed, W_h, b_h, W_o, b_o):
    in_maps = make_in_maps(q_ids, c_ids, num_qs, num_cols, embed, W_h, b_h,
                           W_o, b_o)
    res = run_on_hw(in_maps, trace=False)
    return gather_out(res, b_o)


# revision 37
# speedup vs baseline: 1.0160x; 1.0160x over previous
"""Trainium2 Bass kernel for nn_EntityLinker (ragged_sequence).

Pure data-parallel over batch: 1024 batches -> 8 cores x 128 batches.

Gather strategy: the SWDGE fixed overhead (994ns/call) makes per-pair
indirect DMACopy gathers (576 calls/core) the bottleneck, so instead we use
the custom GPSIMD dma_gather instruction (994ns + 0.34ns/row per call,
<=896 rows/call from the descriptor-ring cap).  dma_gather needs int16
indices, so the host dedups each half-core's referenced embedding rows
(~31K distinct < 32768) into a per-half fp16 table and remaps indices to
table-local int16.  The device still gathers every reference (36MB/core)
through the DMA engines; the host only does O(refs) integer prep plus a
table layout transform.

Token sums over T=8 column tokens are fp16 identity-matmuls accumulating
in PSUM.  Attention + MLP run 2 batches per 128-partition tile, fp16
operands with fp32 PSUM accumulation, elementwise ops merged over duos
(2 pairs) to halve fixed per-op latency.  The whole per-duo dependency
chain is software-pipelined ACROSS group boundaries (engines execute
their streams in order, so a per-group pipeline would drain at each
boundary and fall behind the gather stream).  b_o is added on host.
"""

import sys

if "/opt/trn_rl_repo" not in sys.path:
    sys.path.insert(0, "/opt/trn_rl_repo")

import numpy as np

V, D = 100000, 128
B, Q, C, T = 1024, 64, 64, 8
NCORES = 8
BL = B // NCORES        # 128 batches per core
PAIRS = BL // 2         # 64 pairs (2 batches per 128-partition tile)
GP = 16                 # pairs per group
NG = PAIRS // GP        # 4 groups
ND = GP // 2            # duos per group
NDG = ND * NG           # duos per core
NB = GP + GP * T        # gather blocks per group: q then c
NH = 2                  # table halves per core (2 groups each)
TBL = 32768             # table rows per half (int16-addressable)
NEG = np.float32(-20000.0)   # fp16-safe mask bias
SCALE_SIM = float(1.0 / np.sqrt(128.0))

_P_H = np.arange(128) // 64     # which batch of the pair this partition holds
_P_C = np.arange(128) % 64      # column / q index within the batch


def _wrap16(flat):
    """dma_gather index layout: element i -> partition i%16, col i//16,
    replicated across the 8 GPSIMD sub-cores (128 partitions)."""
    w = flat.reshape(-1, 16).T          # [16, n//16]
    return np.ascontiguousarray(np.tile(w, (8, 1)))  # [128, n//16]


def _prep_core(core, q_ids, c_ids, num_qs, cnt, embed16):
    base = core * BL
    jj = np.arange(GP)

    out = dict()
    for h in range(NH):
        hb = base + h * (BL // NH)            # first batch of the half
        qh = q_ids[hb:hb + BL // NH]          # [64, Q]
        ch = c_ids[hb:hb + BL // NH]          # [64, C, T]
        refs = np.concatenate([qh.ravel(), ch.ravel()])
        uniq, inv = np.unique(refs, return_inverse=True)
        assert len(uniq) <= TBL, f"half table overflow: {len(uniq)}"
        tab = np.zeros((TBL, D), np.float16)
        tab[:len(uniq)] = embed16[uniq]
        out[f"tab{h}"] = tab
        nq = Q * (BL // NH)
        out[f"inv_q{h}"] = inv[:nq].reshape(BL // NH, Q).astype(np.int16)
        out[f"inv_c{h}"] = inv[nq:].reshape(BL // NH, C, T).astype(np.int16)

    q_idx = np.empty((NG * 128, (GP * 128) // 16), np.int16)
    c_idx = np.empty((NG * 128, (GP * T * 128) // 16), np.int16)
    qbias = np.empty((NG * 2, GP * 128), np.float16)
    qv = np.zeros((NG * 128, 2 * GP), np.float16)
    cscale = np.empty((NG * 128, GP), np.float32)

    for g in range(NG):
        h = g // 2
        inv_q = out[f"inv_q{h}"]
        inv_c = out[f"inv_c{h}"]
        # local (within-half) batch index per (partition, pair)
        lb = (g % 2) * 2 * GP + 2 * jj[None, :] + _P_H[:, None]   # [128, GP]
        cm = np.broadcast_to(_P_C[:, None], lb.shape)             # [128, GP]

        # q slots first: i = j*128 + pc
        qi = inv_q[lb, cm]                                        # [128, GP]
        # then c slots duo-major: i = u*2048 + t*256 + p2*128 + pc, so the
        # two pairs of a duo land in adjacent blocks for merged T-sums
        ci = inv_c[lb, cm]                                        # [128, GP, T]
        ci_r = ci.transpose(1, 2, 0).reshape(GP // 2, 2, T, 128)  # u, p2, t, pc
        q_idx[g * 128:(g + 1) * 128] = _wrap16(qi.T.ravel())
        c_idx[g * 128:(g + 1) * 128] = _wrap16(
            ci_r.transpose(0, 2, 1, 3).ravel())

        gbat = base + h * (BL // NH) + lb                         # global batch
        cscale[g * 128:(g + 1) * 128] = 1.0 / cnt[gbat, cm]
        for r in range(2):
            bvec = base + g * 2 * GP + 2 * jj + r
            nqs = num_qs[bvec]
            blk = np.full((GP, 128), NEG, np.float32)
            blk[:, r * 64:(r + 1) * 64] = np.where(
                np.arange(64)[None, :] < nqs[:, None], np.float32(0.0), NEG)
            qbias[g * 2 + r] = blk.reshape(-1).astype(np.float16)
            valid = (_P_C[:, None] < nqs[None, :]) & (_P_H[:, None] == r)
            qv[g * 128:(g + 1) * 128, 2 * jj + r] = \
                (valid / nqs[None, :]).astype(np.float16)

    return dict(tab0=out["tab0"], tab1=out["tab1"], q_idx=q_idx,
                c_idx=c_idx, qbias=qbias, qv=qv, cscale=cscale)


def prep_all(q_ids, c_ids, num_qs, embed):
    q_ids = np.asarray(q_ids).astype(np.int32)
    c_ids = np.asarray(c_ids).astype(np.int32)
    num_qs = np.asarray(num_qs).astype(np.int64)
    cnt = np.maximum((c_ids != 0).sum(-1), 1).astype(np.float32)     # [B, C]
    embed16 = np.asarray(embed, np.float32).astype(np.float16)
    return [_prep_core(i, q_ids, c_ids, num_qs, cnt, embed16)
            for i in range(NCORES)]


_BLOCKIND = np.zeros((2, 128), np.float16)
_BLOCKIND[0, :64] = 1.0
_BLOCKIND[1, 64:] = 1.0


def _build_program():
    from contextlib import ExitStack

    import concourse.bass as bass
    from concourse import bacc, mybir, tile
    from concourse.library_config import mlp
    from concourse.masks import make_identity

    f32 = mybir.dt.float32
    f16 = mybir.dt.float16
    i16 = mybir.dt.int16

    nc = bacc.Bacc("TRN2", target_bir_lowering=False, debug=False,
                   enable_asserts=False, num_devices=NCORES)

    tab0_d = nc.dram_tensor("tab0", [TBL, D], f16, kind="ExternalInput").ap()
    tab1_d = nc.dram_tensor("tab1", [TBL, D], f16, kind="ExternalInput").ap()
    whk_d = nc.dram_tensor("whk", [128, 5 * 128], f16, kind="ExternalInput").ap()
    w_o_d = nc.dram_tensor("w_o", [D, 1], f16, kind="ExternalInput").ap()
    b_h_d = nc.dram_tensor("b_h", [D, 1], f32, kind="ExternalInput").ap()
    blockind_d = nc.dram_tensor("blockind", [2, 128], f16, kind="ExternalInput").ap()
    q_idx_d = nc.dram_tensor("q_idx", [NG * 128, (GP * 128) // 16], i16,
                             kind="ExternalInput").ap()
    c_idx_d = nc.dram_tensor("c_idx", [NG * 128, (GP * T * 128) // 16], i16,
                             kind="ExternalInput").ap()
    qbias_d = nc.dram_tensor("qbias", [NG * 2, GP * 128], f16, kind="ExternalInput").ap()
    qv_d = nc.dram_tensor("qv", [NG * 128, 2 * GP], f16, kind="ExternalInput").ap()
    cscale_d = nc.dram_tensor("cscale", [NG * 128, GP], f32, kind="ExternalInput").ap()
    out_d = nc.dram_tensor("out", [128, PAIRS], f32, kind="ExternalOutput").ap()

    Alu = mybir.AluOpType

    with tile.TileContext(nc) as tc, ExitStack() as ctx:
        const = ctx.enter_context(tc.tile_pool(name="const", bufs=1))
        gpool = ctx.enter_context(tc.tile_pool(name="gather", bufs=3))
        spool = ctx.enter_context(tc.tile_pool(name="work", bufs=5))
        ppool = ctx.enter_context(tc.tile_pool(name="psum", bufs=5, space="PSUM"))
        tpool = ctx.enter_context(tc.tile_pool(name="psumt", bufs=2, space="PSUM"))
        gpsum = ctx.enter_context(tc.tile_pool(name="gps", bufs=1, space="PSUM"))
        hpool = ctx.enter_context(tc.tile_pool(name="hbuf", bufs=ND + 3))

        ident = const.tile([128, 128], f16)
        make_identity(nc, ident[:])
        nc.gpsimd.load_library(mlp)

        whk = const.tile([128, 5 * 128], f16)
        nc.sync.dma_start(whk[:], whk_d[:])
        w_o_t = const.tile([128, 1], f16)
        nc.sync.dma_start(w_o_t[:], w_o_d[:])
        b_h_t = const.tile([128, 1], f32)
        nc.sync.dma_start(b_h_t[:], b_h_d[:])
        blockind_t = const.tile([2, 128], f16)
        nc.sync.dma_start(blockind_t[:], blockind_d[:])
        out_sb = const.tile([128, PAIRS], f32)

        Act = mybir.ActivationFunctionType

        groups = [dict() for _ in range(NG)]
        st = [dict() for _ in range(NDG)]

        def tab_of(g):
            return tab0_d if g < NG // 2 else tab1_d

        def emit_q_io(g):
            """q gathers + mask loads (issued >=1 group ahead of the group's
            compute so the prologue's qs matmuls never stall the in-order
            engine streams waiting on fresh gather data)."""
            G = groups[g]
            qidx_t = gpool.tile([128, (GP * 128) // 16], i16, tag="qidx",
                                name="qidx", bufs=NG)
            G["qbias"] = gpool.tile([2, GP * 128], f16, tag="qbias",
                                    name="qbias", bufs=NG)
            G["qv"] = gpool.tile([128, 2 * GP], f16, tag="qv", name="qv",
                                 bufs=NG)
            G["csc"] = gpool.tile([128, GP], f32, tag="csc", name="csc",
                                  bufs=NG)
            nc.sync.dma_start(qidx_t[:], q_idx_d[g * 128:(g + 1) * 128, :])
            nc.sync.dma_start(G["qbias"][:], qbias_d[g * 2:g * 2 + 2, :])
            nc.sync.dma_start(G["qv"][:], qv_d[g * 128:(g + 1) * 128, :])
            nc.sync.dma_start(G["csc"][:], cscale_d[g * 128:(g + 1) * 128, :])
            qdst = gpool.tile([128, GP, 128], f16, tag="qdst", name="qdst",
                              bufs=NG)
            G["qdst"] = qdst
            s = 0
            while s < GP:
                m = min(7, GP - s)
                nc.gpsimd.dma_gather(qdst[:, s:s + m, :], tab_of(g)[:],
                                     qidx_t[:, s * 8:(s + m) * 8],
                                     m * 128, m * 128, D)
                s += m

        def emit_group_io(g, upto=GP * T):
            """Chunked dma_gather of the group's c tokens (duo-major).
            ``upto`` allows splitting the chunk stream so startup q gathers
            can interleave after the first few c chunks."""
            G = groups[g]
            if "cidx" not in G:
                G["cidx"] = gpool.tile([128, (GP * T * 128) // 16], i16,
                                       tag="cidx", name="cidx")
                nc.sync.dma_start(G["cidx"][:],
                                  c_idx_d[g * 128:(g + 1) * 128, :])
                G["dest"] = gpool.tile([128, GP * T, 128], f16, tag="dst",
                                       name="dest")
                G["c_done"] = 0
            dest, cidx_t = G["dest"], G["cidx"]
            s = G["c_done"]
            while s < upto:
                m = min(7, GP * T - s)
                nc.gpsimd.dma_gather(dest[:, s:s + m, :], tab_of(g)[:],
                                     cidx_t[:, s * 8:(s + m) * 8],
                                     m * 128, m * 128, D)
                s += m
            G["c_done"] = s

        # Prologue (q_summary -> per-batch MLP bias columns, transposed to
        # biasT[r, j*128+d]) is split into three stages emitted at the END
        # of successive iterations so its data wait on the fresh q gather
        # never head-of-line-blocks the per-duo pipeline.
        def prologue_1(g):
            G = groups[g]
            qdst = G["qdst"]
            qs_psA = ppool.tile([128, 512], f32, tag="psA", name="qs_psA")
            qs_ps = qs_psA[:, 0:2 * GP]
            for j in range(GP):
                nc.tensor.matmul(qs_ps[:, 2 * j:2 * j + 2],
                                 lhsT=qdst[:, j, :],
                                 rhs=G["qv"][:, 2 * j:2 * j + 2],
                                 start=True, stop=True)
            G["qs_sb"] = spool.tile([128, 2 * GP], f16, tag="qs_sb",
                                    name="qs_sb")
            nc.vector.tensor_copy(G["qs_sb"][:], qs_ps)

        def prologue_2(g):
            G = groups[g]
            bias_psA = ppool.tile([128, 512], f32, tag="psA", name="bias_psA")
            bias_ps = bias_psA[:, 0:2 * GP]
            nc.tensor.matmul(bias_ps, lhsT=whk[:, 0:128], rhs=G["qs_sb"][:],
                             start=True, stop=True)
            G["bias_sb"] = spool.tile([128, 2 * GP], f16, tag="bias_sb",
                                      name="bias_sb")
            nc.scalar.activation(G["bias_sb"][:], bias_ps, Act.Identity,
                                 bias=b_h_t[:, 0:1])

        def prologue_3(g):
            G = groups[g]
            bias_sb = G["bias_sb"]
            biasT = spool.tile([2, GP * 128], f16, tag="biasT", name="biasT")
            G["biasT"] = biasT
            for quarter in range(4):
                bt_ps = tpool.tile([128, 768], f16, tag="pst", name="bt_ps")
                for jj_ in range(4):
                    j = quarter * 4 + jj_
                    nc.tensor.transpose(bt_ps[0:2, jj_ * 128:(jj_ + 1) * 128],
                                        bias_sb[:, 2 * j:2 * j + 2], ident[:])
                nc.vector.tensor_copy(
                    biasT[:, quarter * 512:(quarter + 1) * 512],
                    bt_ps[0:2, 0:512])

        def phase_a(u):
            g, ul = u // ND, u % ND
            G, s = groups[g], st[u]
            s["psA1"] = ppool.tile([128, 512], f32, tag="psA", name="psA1")
            cs = s["psA1"][:, 0:256]
            for t in range(T):
                b = ul * 2 * T + t * 2
                nc.tensor.matmul(cs, lhsT=ident[:],
                                 rhs=G["dest"][:, b:b + 2, :],
                                 start=(t == 0), stop=(t == T - 1))
            s["c_h2"] = spool.tile([128, 256], f16, tag="c_h2", name="c_h2")
            for p in range(2):
                nc.vector.tensor_scalar_mul(
                    s["c_h2"][:, p * 128:(p + 1) * 128],
                    s["psA1"][:, p * 128:(p + 1) * 128],
                    G["csc"][:, 2 * ul + p:2 * ul + p + 1])

        def phase_b_pre(u):
            g, ul = u // ND, u % ND
            G, s = groups[g], st[u]
            s["psT"] = tpool.tile([128, 768], f16, tag="pst", name="psT")
            for p in range(2):
                nc.tensor.transpose(s["psT"][:, p * 256:p * 256 + 128],
                                    s["c_h2"][:, p * 128:(p + 1) * 128],
                                    ident[:])
                nc.tensor.transpose(s["psT"][:, p * 256 + 128:p * 256 + 256],
                                    G["qdst"][:, 2 * ul + p, :], ident[:])

        def phase_b_post(u):
            g, ul = u // ND, u % ND
            G, s = groups[g], st[u]
            s["cq_hT"] = spool.tile([128, 512], f16, tag="cq_hT", name="cq_hT")
            nc.scalar.copy(s["cq_hT"][:], s["psT"][:, 0:512])
            for p in range(2):
                j = 2 * ul + p
                sim = s["psA1"][:, 256 + p * 128:256 + (p + 1) * 128]
                nc.tensor.matmul(sim, lhsT=s["cq_hT"][:, p * 256:p * 256 + 128],
                                 rhs=s["cq_hT"][:, p * 256 + 128:p * 256 + 256],
                                 start=True, stop=False)
                nc.tensor.matmul(sim, lhsT=blockind_t[:],
                                 rhs=G["qbias"][:, j * 128:(j + 1) * 128],
                                 start=False, stop=True)
            s["att_e"] = spool.tile([128, 256], f16, tag="att_e", name="att_e")
            nc.scalar.activation(s["att_e"][:], s["psA1"][:, 256:512],
                                 Act.Exp, scale=SCALE_SIM)

        def phase_c1(u):
            g, ul = u // ND, u % ND
            G, s = groups[g], st[u]
            s_cols = spool.tile([128, 2], f32, tag="s_cols", name="s_cols")
            nc.vector.tensor_reduce(
                s_cols[:],
                s["att_e"][:].rearrange("p (x d) -> p x d", d=128),
                axis=mybir.AxisListType.X, op=Alu.add)
            r_cols = spool.tile([128, 2], f32, tag="r_cols", name="r_cols")
            nc.vector.reciprocal(r_cols[:], s_cols[:])
            att = spool.tile([128, 256], f16, tag="att", name="att")
            for p in range(2):
                nc.vector.tensor_scalar_mul(
                    att[:, p * 128:(p + 1) * 128],
                    s["att_e"][:, p * 128:(p + 1) * 128],
                    r_cols[:, p:p + 1])
                nc.tensor.transpose(
                    s["psT"][:, 512 + p * 128:512 + (p + 1) * 128],
                    att[:, p * 128:(p + 1) * 128], ident[:])
            s["attT"] = spool.tile([128, 256], f16, tag="attT", name="attT")
            nc.scalar.copy(s["attT"][:], s["psT"][:, 512:768])

        def phase_c2(u):
            g, ul = u // ND, u % ND
            G, s = groups[g], st[u]
            for p in range(2):
                nc.tensor.matmul(s["psA1"][:, p * 128:(p + 1) * 128],
                                 lhsT=G["qdst"][:, 2 * ul + p, :],
                                 rhs=s["attT"][:, p * 128:(p + 1) * 128],
                                 start=True, stop=True)
            wqT = spool.tile([128, 256], f16, tag="wqT", name="wqT")
            s["wqT"] = wqT
            nc.vector.tensor_copy(wqT[:], s["psA1"][:, 0:256])
            ch3 = spool.tile([128, 256], f16, tag="ch3", name="ch3")
            dif = spool.tile([128, 256], f16, tag="dif", name="dif")
            for p in range(2):
                c_hT = s["cq_hT"][:, p * 256:p * 256 + 128]
                wqT_p = wqT[:, p * 128:(p + 1) * 128]
                nc.vector.tensor_mul(ch3[:, p * 128:(p + 1) * 128], c_hT, wqT_p)
                nc.vector.tensor_sub(dif[:, p * 128:(p + 1) * 128], c_hT, wqT_p)
            s["ch3"] = ch3
            ch4 = spool.tile([128, 256], f16, tag="ch4", name="ch4")
            nc.scalar.activation(ch4[:], dif[:], Act.Abs)
            s["ch4"] = ch4

        def phase_d(u):
            g, ul = u // ND, u % ND
            G, s = groups[g], st[u]
            for p in range(2):
                j = 2 * ul + p
                h = s["psA1"][:, 256 + p * 128:256 + (p + 1) * 128]
                c_hT = s["cq_hT"][:, p * 256:p * 256 + 128]
                sl = slice(p * 128, (p + 1) * 128)
                for k, rhs in ((1, c_hT), (2, s["wqT"][:, sl]),
                               (3, s["ch3"][:, sl]), (4, s["ch4"][:, sl])):
                    nc.tensor.matmul(h, lhsT=whk[:, k * 128:(k + 1) * 128],
                                     rhs=rhs, start=(k == 1), stop=False)
                nc.tensor.matmul(h, lhsT=G["biasT"][:, j * 128:(j + 1) * 128],
                                 rhs=blockind_t[:], start=False, stop=True)
            s["hT"] = hpool.tile([128, 256], f16, tag="hT", name="hT")
            nc.scalar.activation(s["hT"][:], s["psA1"][:, 256:512], Act.Tanh)

        def emit_epilogue(g):
            out_ps = gpsum.tile([128, GP], f32, tag="outp", name="out_ps")
            for ul in range(ND):
                s = st[g * ND + ul]
                for p in range(2):
                    nc.tensor.matmul(
                        out_ps[:, 2 * ul + p:2 * ul + p + 1],
                        lhsT=s["hT"][:, p * 128:(p + 1) * 128],
                        rhs=w_o_t[:], start=True, stop=True)
            nc.scalar.activation(out_sb[:, g * GP:(g + 1) * GP], out_ps[:],
                                 Act.Identity)

        # Global software pipeline: phases staggered across ALL duos so no
        # engine stream drains at group boundaries.  Within an iteration the
        # OLDEST phases are emitted first: engines execute their streams in
        # order, so putting the newest (gather-data-dependent) ops first
        # would head-of-line-block ready work for older duos.
        emit_q_io(0)
        emit_q_io(1)
        for i in range(NDG + 6):
            if i < NDG and i % ND == 0:
                g = i // ND
                emit_group_io(g)
                if g + 2 < NG:
                    emit_q_io(g + 2)
            if 5 <= i and i - 5 < NDG:
                phase_d(i - 5)
            if 4 <= i and i - 4 < NDG:
                phase_c2(i - 4)
            if 3 <= i and i - 3 < NDG:
                phase_c1(i - 3)
            if 2 <= i and i - 2 < NDG:
                phase_b_post(i - 2)
            if 1 <= i and i - 1 < NDG:
                phase_b_pre(i - 1)
            if i < NDG:
                phase_a(i)
            if 1 <= i and (i - 1) % ND == 0 and (i - 1) // ND < NG:
                prologue_1((i - 1) // ND)
            if 2 <= i and (i - 2) % ND == 0 and (i - 2) // ND < NG:
                prologue_2((i - 2) // ND)
            if 3 <= i and (i - 3) % ND == 0 and (i - 3) // ND < NG:
                prologue_3((i - 3) // ND)
            if i >= ND + 4 and (i - ND - 4) % ND == 0 and (i - ND - 4) // ND < NG:
                emit_epilogue((i - ND - 4) // ND)

        nc.sync.dma_start(out_d[:], out_sb[:])

    nc.compile()
    return nc


_PROGRAM = None


def _get_program():
    global _PROGRAM
    if _PROGRAM is None:
        _PROGRAM = _build_program()
    return _PROGRAM


def run_on_hw(in_maps, trace=False, **kw):
    from concourse import bass_utils
    nc = _get_program()
    return bass_utils.run_bass_kernel_spmd(
        nc, in_maps, core_ids=list(range(NCORES)), trace=trace, **kw)


def make_in_maps(q_ids, c_ids, num_qs, num_cols, embed, W_h, b_h, W_o, b_o):
    W_h = np.asarray(W_h, np.float32)
    whk = np.ascontiguousarray(
        W_h.reshape(5, 128, 128).transpose(1, 0, 2).reshape(128, 5 * 128)
    ).astype(np.float16)
    w_o = np.ascontiguousarray(
        np.asarray(W_o, np.float32).reshape(D, 1)).astype(np.float16)
    b_h_col = np.ascontiguousarray(
        np.asarray(b_h, np.float32).reshape(D, 1))
    shared = dict(whk=whk, w_o=w_o, b_h=b_h_col, blockind=_BLOCKIND)
    percore = prep_all(q_ids, c_ids, num_qs, embed)
    return [dict(shared, **percore[i]) for i in range(NCORES)]


def gather_out(res, b_o):
    b_o_val = np.float32(np.asarray(b_o).reshape(-1)[0])
    outs = np.empty((B, C, 1), np.float32)
    for i in range(NCORES):
        o = np.asarray(res.results[i]["out"], np.float32)  # [pc, j]
        # pc = 64*r + col ; batch = i*BL + 2*j + r
        o = o.reshape(2, 64, PAIRS)          # [r, col, j]
        o = o.transpose(2, 0, 1).reshape(BL, C)   # [(j, r), col]
        outs[i * BL:(i + 1) * BL, :, 0] = o + b_o_val
    return outs


def kernel(q_ids, c_ids, num_qs, num_cols, embed, W_h, b_h, W_o, b_o):
    in_maps = make_in_maps(q_ids, c_ids, num_qs, num_cols, embed, W_h, b_h,
                           W_o, b_o)
    res = run_on_hw(in_maps, trace=False)
    return gather_out(res, b_o)


# revision 38
# speedup vs baseline: 1.0180x; 1.0020x over previous
"""Trainium2 Bass kernel for nn_EntityLinker (ragged_sequence).

Pure data-parallel over batch: 1024 batches -> 8 cores x 128 batches.

Gather strategy: the SWDGE fixed overhead (994ns/call) makes per-pair
indirect DMACopy gathers (576 calls/core) the bottleneck, so instead we use
the custom GPSIMD dma_gather instruction (994ns + 0.34ns/row per call,
<=896 rows/call from the descriptor-ring cap).  dma_gather needs int16
indices, so the host dedups each half-core's referenced embedding rows
(~31K distinct < 32768) into a per-half fp16 table and remaps indices to
table-local int16.  The device still gathers every reference (36MB/core)
through the DMA engines; the host only does O(refs) integer prep plus a
table layout transform.

Token sums over T=8 column tokens are fp16 identity-matmuls accumulating
in PSUM.  Attention + MLP run 2 batches per 128-partition tile, fp16
operands with fp32 PSUM accumulation, elementwise ops merged over duos
(2 pairs) to halve fixed per-op latency.  The whole per-duo dependency
chain is software-pipelined ACROSS group boundaries (engines execute
their streams in order, so a per-group pipeline would drain at each
boundary and fall behind the gather stream).  b_o is added on host.
"""

import sys

if "/opt/trn_rl_repo" not in sys.path:
    sys.path.insert(0, "/opt/trn_rl_repo")

import numpy as np

V, D = 100000, 128
B, Q, C, T = 1024, 64, 64, 8
NCORES = 8
BL = B // NCORES        # 128 batches per core
PAIRS = BL // 2         # 64 pairs (2 batches per 128-partition tile)
GP = 16                 # pairs per group
NG = PAIRS // GP        # 4 groups
ND = GP // 2            # duos per group
NDG = ND * NG           # duos per core
NB = GP + GP * T        # gather blocks per group: q then c
NH = 2                  # table halves per core (2 groups each)
TBL = 32768             # table rows per half (int16-addressable)
NEG = np.float32(-20000.0)   # fp16-safe mask bias
SCALE_SIM = float(1.0 / np.sqrt(128.0))

_P_H = np.arange(128) // 64     # which batch of the pair this partition holds
_P_C = np.arange(128) % 64      # column / q index within the batch


def _wrap16(flat):
    """dma_gather index layout: element i -> partition i%16, col i//16,
    replicated across the 8 GPSIMD sub-cores (128 partitions)."""
    w = flat.reshape(-1, 16).T          # [16, n//16]
    return np.ascontiguousarray(np.tile(w, (8, 1)))  # [128, n//16]


def _prep_core(core, q_ids, c_ids, num_qs, cnt, embed16):
    base = core * BL
    jj = np.arange(GP)

    out = dict()
    for h in range(NH):
        hb = base + h * (BL // NH)            # first batch of the half
        qh = q_ids[hb:hb + BL // NH]          # [64, Q]
        ch = c_ids[hb:hb + BL // NH]          # [64, C, T]
        refs = np.concatenate([qh.ravel(), ch.ravel()])
        uniq, inv = np.unique(refs, return_inverse=True)
        assert len(uniq) <= TBL, f"half table overflow: {len(uniq)}"
        tab = np.zeros((TBL, D), np.float16)
        tab[:len(uniq)] = embed16[uniq]
        out[f"tab{h}"] = tab
        nq = Q * (BL // NH)
        out[f"inv_q{h}"] = inv[:nq].reshape(BL // NH, Q).astype(np.int16)
        out[f"inv_c{h}"] = inv[nq:].reshape(BL // NH, C, T).astype(np.int16)

    q_idx = np.empty((NG * 128, (GP * 128) // 16), np.int16)
    c_idx = np.empty((NG * 128, (GP * T * 128) // 16), np.int16)
    qbias = np.empty((NG * 2, GP * 128), np.float16)
    qv = np.zeros((NG * 128, 2 * GP), np.float16)
    cscale = np.empty((NG * 128, GP), np.float32)

    for g in range(NG):
        h = g // 2
        inv_q = out[f"inv_q{h}"]
        inv_c = out[f"inv_c{h}"]
        # local (within-half) batch index per (partition, pair)
        lb = (g % 2) * 2 * GP + 2 * jj[None, :] + _P_H[:, None]   # [128, GP]
        cm = np.broadcast_to(_P_C[:, None], lb.shape)             # [128, GP]

        # q slots first: i = j*128 + pc
        qi = inv_q[lb, cm]                                        # [128, GP]
        # then c slots duo-major: i = u*2048 + t*256 + p2*128 + pc, so the
        # two pairs of a duo land in adjacent blocks for merged T-sums
        ci = inv_c[lb, cm]                                        # [128, GP, T]
        ci_r = ci.transpose(1, 2, 0).reshape(GP // 2, 2, T, 128)  # u, p2, t, pc
        q_idx[g * 128:(g + 1) * 128] = _wrap16(qi.T.ravel())
        c_idx[g * 128:(g + 1) * 128] = _wrap16(
            ci_r.transpose(0, 2, 1, 3).ravel())

        gbat = base + h * (BL // NH) + lb                         # global batch
        cscale[g * 128:(g + 1) * 128] = 1.0 / cnt[gbat, cm]
        for r in range(2):
            bvec = base + g * 2 * GP + 2 * jj + r
            nqs = num_qs[bvec]
            blk = np.full((GP, 128), NEG, np.float32)
            blk[:, r * 64:(r + 1) * 64] = np.where(
                np.arange(64)[None, :] < nqs[:, None], np.float32(0.0), NEG)
            qbias[g * 2 + r] = blk.reshape(-1).astype(np.float16)
            valid = (_P_C[:, None] < nqs[None, :]) & (_P_H[:, None] == r)
            qv[g * 128:(g + 1) * 128, 2 * jj + r] = \
                (valid / nqs[None, :]).astype(np.float16)

    return dict(tab0=out["tab0"], tab1=out["tab1"], q_idx=q_idx,
                c_idx=c_idx, qbias=qbias, qv=qv, cscale=cscale)


def prep_all(q_ids, c_ids, num_qs, embed):
    q_ids = np.asarray(q_ids).astype(np.int32)
    c_ids = np.asarray(c_ids).astype(np.int32)
    num_qs = np.asarray(num_qs).astype(np.int64)
    cnt = np.maximum((c_ids != 0).sum(-1), 1).astype(np.float32)     # [B, C]
    embed16 = np.asarray(embed, np.float32).astype(np.float16)
    return [_prep_core(i, q_ids, c_ids, num_qs, cnt, embed16)
            for i in range(NCORES)]


_BLOCKIND = np.zeros((2, 128), np.float16)
_BLOCKIND[0, :64] = 1.0
_BLOCKIND[1, 64:] = 1.0


def _build_program():
    from contextlib import ExitStack

    import concourse.bass as bass
    from concourse import bacc, mybir, tile
    from concourse.library_config import mlp
    from concourse.masks import make_identity

    f32 = mybir.dt.float32
    f16 = mybir.dt.float16
    i16 = mybir.dt.int16

    nc = bacc.Bacc("TRN2", target_bir_lowering=False, debug=False,
                   enable_asserts=False, num_devices=NCORES)

    tab0_d = nc.dram_tensor("tab0", [TBL, D], f16, kind="ExternalInput").ap()
    tab1_d = nc.dram_tensor("tab1", [TBL, D], f16, kind="ExternalInput").ap()
    whk_d = nc.dram_tensor("whk", [128, 5 * 128], f16, kind="ExternalInput").ap()
    w_o_d = nc.dram_tensor("w_o", [D, 1], f16, kind="ExternalInput").ap()
    b_h_d = nc.dram_tensor("b_h", [D, 1], f32, kind="ExternalInput").ap()
    blockind_d = nc.dram_tensor("blockind", [2, 128], f16, kind="ExternalInput").ap()
    q_idx_d = nc.dram_tensor("q_idx", [NG * 128, (GP * 128) // 16], i16,
                             kind="ExternalInput").ap()
    c_idx_d = nc.dram_tensor("c_idx", [NG * 128, (GP * T * 128) // 16], i16,
                             kind="ExternalInput").ap()
    qbias_d = nc.dram_tensor("qbias", [NG * 2, GP * 128], f16, kind="ExternalInput").ap()
    qv_d = nc.dram_tensor("qv", [NG * 128, 2 * GP], f16, kind="ExternalInput").ap()
    cscale_d = nc.dram_tensor("cscale", [NG * 128, GP], f32, kind="ExternalInput").ap()
    out_d = nc.dram_tensor("out", [128, PAIRS], f32, kind="ExternalOutput").ap()

    Alu = mybir.AluOpType

    with tile.TileContext(nc) as tc, ExitStack() as ctx:
        const = ctx.enter_context(tc.tile_pool(name="const", bufs=1))
        gpool = ctx.enter_context(tc.tile_pool(name="gather", bufs=3))
        spool = ctx.enter_context(tc.tile_pool(name="work", bufs=5))
        ppool = ctx.enter_context(tc.tile_pool(name="psum", bufs=5, space="PSUM"))
        tpool = ctx.enter_context(tc.tile_pool(name="psumt", bufs=2, space="PSUM"))
        gpsum = ctx.enter_context(tc.tile_pool(name="gps", bufs=1, space="PSUM"))
        hpool = ctx.enter_context(tc.tile_pool(name="hbuf", bufs=ND + 3))

        ident = const.tile([128, 128], f16)
        make_identity(nc, ident[:])
        nc.gpsimd.load_library(mlp)

        whk = const.tile([128, 5 * 128], f16)
        nc.sync.dma_start(whk[:], whk_d[:])
        w_o_t = const.tile([128, 1], f16)
        nc.sync.dma_start(w_o_t[:], w_o_d[:])
        b_h_t = const.tile([128, 1], f32)
        nc.sync.dma_start(b_h_t[:], b_h_d[:])
        blockind_t = const.tile([2, 128], f16)
        nc.sync.dma_start(blockind_t[:], blockind_d[:])
        out_sb = const.tile([128, PAIRS], f32)

        Act = mybir.ActivationFunctionType

        groups = [dict() for _ in range(NG)]
        st = [dict() for _ in range(NDG)]

        def tab_of(g):
            return tab0_d if g < NG // 2 else tab1_d

        def emit_q_io(g):
            """q gathers + mask loads (issued >=1 group ahead of the group's
            compute so the prologue's qs matmuls never stall the in-order
            engine streams waiting on fresh gather data)."""
            G = groups[g]
            qidx_t = gpool.tile([128, (GP * 128) // 16], i16, tag="qidx",
                                name="qidx", bufs=NG)
            G["qbias"] = gpool.tile([2, GP * 128], f16, tag="qbias",
                                    name="qbias", bufs=NG)
            G["qv"] = gpool.tile([128, 2 * GP], f16, tag="qv", name="qv",
                                 bufs=NG)
            G["csc"] = gpool.tile([128, GP], f32, tag="csc", name="csc",
                                  bufs=NG)
            nc.sync.dma_start(qidx_t[:], q_idx_d[g * 128:(g + 1) * 128, :])
            nc.sync.dma_start(G["qbias"][:], qbias_d[g * 2:g * 2 + 2, :])
            nc.sync.dma_start(G["qv"][:], qv_d[g * 128:(g + 1) * 128, :])
            nc.sync.dma_start(G["csc"][:], cscale_d[g * 128:(g + 1) * 128, :])
            qdst = gpool.tile([128, GP, 128], f16, tag="qdst", name="qdst",
                              bufs=NG)
            G["qdst"] = qdst
            s = 0
            while s < GP:
                m = min(7, GP - s)
                nc.gpsimd.dma_gather(qdst[:, s:s + m, :], tab_of(g)[:],
                                     qidx_t[:, s * 8:(s + m) * 8],
                                     m * 128, m * 128, D)
                s += m

        def emit_group_io(g, upto=GP * T):
            """Chunked dma_gather of the group's c tokens (duo-major).
            ``upto`` allows splitting the chunk stream so startup q gathers
            can interleave after the first few c chunks."""
            G = groups[g]
            if "cidx" not in G:
                G["cidx"] = gpool.tile([128, (GP * T * 128) // 16], i16,
                                       tag="cidx", name="cidx")
                nc.sync.dma_start(G["cidx"][:],
                                  c_idx_d[g * 128:(g + 1) * 128, :])
                G["dest"] = gpool.tile([128, GP * T, 128], f16, tag="dst",
                                       name="dest")
                G["c_done"] = 0
            dest, cidx_t = G["dest"], G["cidx"]
            s = G["c_done"]
            while s < upto:
                m = min(7, GP * T - s)
                nc.gpsimd.dma_gather(dest[:, s:s + m, :], tab_of(g)[:],
                                     cidx_t[:, s * 8:(s + m) * 8],
                                     m * 128, m * 128, D)
                s += m
            G["c_done"] = s

        # Prologue (q_summary -> per-batch MLP bias columns, transposed to
        # biasT[r, j*128+d]) is split into three stages emitted at the END
        # of successive iterations so its data wait on the fresh q gather
        # never head-of-line-blocks the per-duo pipeline.
        def prologue_1(g):
            G = groups[g]
            qdst = G["qdst"]
            qs_psA = ppool.tile([128, 512], f32, tag="psA", name="qs_psA")
            qs_ps = qs_psA[:, 0:2 * GP]
            for j in range(GP):
                nc.tensor.matmul(qs_ps[:, 2 * j:2 * j + 2],
                                 lhsT=qdst[:, j, :],
                                 rhs=G["qv"][:, 2 * j:2 * j + 2],
                                 start=True, stop=True)
            G["qs_sb"] = spool.tile([128, 2 * GP], f16, tag="qs_sb",
                                    name="qs_sb")
            nc.vector.tensor_copy(G["qs_sb"][:], qs_ps)

        def prologue_2(g):
            G = groups[g]
            bias_psA = ppool.tile([128, 512], f32, tag="psA", name="bias_psA")
            bias_ps = bias_psA[:, 0:2 * GP]
            nc.tensor.matmul(bias_ps, lhsT=whk[:, 0:128], rhs=G["qs_sb"][:],
                             start=True, stop=True)
            G["bias_sb"] = spool.tile([128, 2 * GP], f16, tag="bias_sb",
                                      name="bias_sb")
            nc.scalar.activation(G["bias_sb"][:], bias_ps, Act.Identity,
                                 bias=b_h_t[:, 0:1])

        def prologue_3(g):
            G = groups[g]
            bias_sb = G["bias_sb"]
            biasT = spool.tile([2, GP * 128], f16, tag="biasT", name="biasT")
            G["biasT"] = biasT
            for quarter in range(4):
                bt_ps = tpool.tile([128, 768], f16, tag="pst", name="bt_ps")
                for jj_ in range(4):
                    j = quarter * 4 + jj_
                    nc.tensor.transpose(bt_ps[0:2, jj_ * 128:(jj_ + 1) * 128],
                                        bias_sb[:, 2 * j:2 * j + 2], ident[:])
                nc.vector.tensor_copy(
                    biasT[:, quarter * 512:(quarter + 1) * 512],
                    bt_ps[0:2, 0:512])

        def phase_a(u):
            g, ul = u // ND, u % ND
            G, s = groups[g], st[u]
            s["psA1"] = ppool.tile([128, 512], f32, tag="psA", name="psA1")
            cs = s["psA1"][:, 0:256]
            for t in range(T):
                b = ul * 2 * T + t * 2
                nc.tensor.matmul(cs, lhsT=ident[:],
                                 rhs=G["dest"][:, b:b + 2, :],
                                 start=(t == 0), stop=(t == T - 1))
            s["c_h2"] = spool.tile([128, 256], f16, tag="c_h2", name="c_h2")
            for p in range(2):
                nc.vector.tensor_scalar_mul(
                    s["c_h2"][:, p * 128:(p + 1) * 128],
                    s["psA1"][:, p * 128:(p + 1) * 128],
                    G["csc"][:, 2 * ul + p:2 * ul + p + 1])

        def phase_b_pre(u):
            g, ul = u // ND, u % ND
            G, s = groups[g], st[u]
            s["psT"] = tpool.tile([128, 768], f16, tag="pst", name="psT")
            for p in range(2):
                nc.tensor.transpose(s["psT"][:, p * 256:p * 256 + 128],
                                    s["c_h2"][:, p * 128:(p + 1) * 128],
                                    ident[:])
                nc.tensor.transpose(s["psT"][:, p * 256 + 128:p * 256 + 256],
                                    G["qdst"][:, 2 * ul + p, :], ident[:])

        def phase_b_post(u):
            g, ul = u // ND, u % ND
            G, s = groups[g], st[u]
            s["cq_hT"] = spool.tile([128, 512], f16, tag="cq_hT", name="cq_hT")
            nc.scalar.copy(s["cq_hT"][:], s["psT"][:, 0:512])
            for p in range(2):
                j = 2 * ul + p
                sim = s["psA1"][:, 256 + p * 128:256 + (p + 1) * 128]
                nc.tensor.matmul(sim, lhsT=s["cq_hT"][:, p * 256:p * 256 + 128],
                                 rhs=s["cq_hT"][:, p * 256 + 128:p * 256 + 256],
                                 start=True, stop=False)
                nc.tensor.matmul(sim, lhsT=blockind_t[:],
                                 rhs=G["qbias"][:, j * 128:(j + 1) * 128],
                                 start=False, stop=True)
            s["att_e"] = spool.tile([128, 256], f16, tag="att_e", name="att_e")
            nc.scalar.activation(s["att_e"][:], s["psA1"][:, 256:512],
                                 Act.Exp, scale=SCALE_SIM)

        def phase_c1(u):
            g, ul = u // ND, u % ND
            G, s = groups[g], st[u]
            s_cols = spool.tile([128, 2], f32, tag="s_cols", name="s_cols")
            nc.vector.tensor_reduce(
                s_cols[:],
                s["att_e"][:].rearrange("p (x d) -> p x d", d=128),
                axis=mybir.AxisListType.X, op=Alu.add)
            r_cols = spool.tile([128, 2], f32, tag="r_cols", name="r_cols")
            nc.vector.reciprocal(r_cols[:], s_cols[:])
            att = spool.tile([128, 256], f16, tag="att", name="att")
            for p in range(2):
                nc.vector.tensor_scalar_mul(
                    att[:, p * 128:(p + 1) * 128],
                    s["att_e"][:, p * 128:(p + 1) * 128],
                    r_cols[:, p:p + 1])
                nc.tensor.transpose(
                    s["psT"][:, 512 + p * 128:512 + (p + 1) * 128],
                    att[:, p * 128:(p + 1) * 128], ident[:])
            s["attT"] = spool.tile([128, 256], f16, tag="attT", name="attT")
            nc.scalar.copy(s["attT"][:], s["psT"][:, 512:768])

        def phase_c2(u):
            g, ul = u // ND, u % ND
            G, s = groups[g], st[u]
            for p in range(2):
                nc.tensor.matmul(s["psA1"][:, p * 128:(p + 1) * 128],
                                 lhsT=G["qdst"][:, 2 * ul + p, :],
                                 rhs=s["attT"][:, p * 128:(p + 1) * 128],
                                 start=True, stop=True)
            wqT = spool.tile([128, 256], f16, tag="wqT", name="wqT")
            s["wqT"] = wqT
            nc.vector.tensor_copy(wqT[:], s["psA1"][:, 0:256])
            ch3 = spool.tile([128, 256], f16, tag="ch3", name="ch3")
            dif = spool.tile([128, 256], f16, tag="dif", name="dif")
            for p in range(2):
                c_hT = s["cq_hT"][:, p * 256:p * 256 + 128]
                wqT_p = wqT[:, p * 128:(p + 1) * 128]
                nc.vector.tensor_mul(ch3[:, p * 128:(p + 1) * 128], c_hT, wqT_p)
                nc.vector.tensor_sub(dif[:, p * 128:(p + 1) * 128], c_hT, wqT_p)
            s["ch3"] = ch3
            ch4 = spool.tile([128, 256], f16, tag="ch4", name="ch4")
            nc.scalar.activation(ch4[:], dif[:], Act.Abs)
            s["ch4"] = ch4

        def phase_d(u):
            g, ul = u // ND, u % ND
            G, s = groups[g], st[u]
            for p in range(2):
                j = 2 * ul + p
                h = s["psA1"][:, 256 + p * 128:256 + (p + 1) * 128]
                c_hT = s["cq_hT"][:, p * 256:p * 256 + 128]
                sl = slice(p * 128, (p + 1) * 128)
                for k, rhs in ((1, c_hT), (2, s["wqT"][:, sl]),
                               (3, s["ch3"][:, sl]), (4, s["ch4"][:, sl])):
                    nc.tensor.matmul(h, lhsT=whk[:, k * 128:(k + 1) * 128],
                                     rhs=rhs, start=(k == 1), stop=False)
                nc.tensor.matmul(h, lhsT=G["biasT"][:, j * 128:(j + 1) * 128],
                                 rhs=blockind_t[:], start=False, stop=True)
            s["hT"] = hpool.tile([128, 256], f16, tag="hT", name="hT")
            nc.scalar.activation(s["hT"][:], s["psA1"][:, 256:512], Act.Tanh)

        def emit_epilogue(g):
            out_ps = gpsum.tile([128, GP], f32, tag="outp", name="out_ps")
            for ul in range(ND):
                s = st[g * ND + ul]
                for p in range(2):
                    nc.tensor.matmul(
                        out_ps[:, 2 * ul + p:2 * ul + p + 1],
                        lhsT=s["hT"][:, p * 128:(p + 1) * 128],
                        rhs=w_o_t[:], start=True, stop=True)
            nc.scalar.activation(out_sb[:, g * GP:(g + 1) * GP], out_ps[:],
                                 Act.Identity)
            nc.sync.dma_start(out_d[:, g * GP:(g + 1) * GP],
                              out_sb[:, g * GP:(g + 1) * GP])

        # Global software pipeline: phases staggered across ALL duos so no
        # engine stream drains at group boundaries.  Within an iteration the
        # OLDEST phases are emitted first: engines execute their streams in
        # order, so putting the newest (gather-data-dependent) ops first
        # would head-of-line-block ready work for older duos.
        emit_q_io(0)
        emit_q_io(1)
        for i in range(NDG + 6):
            if i < NDG and i % ND == 0:
                g = i // ND
                emit_group_io(g)
                if g + 2 < NG:
                    emit_q_io(g + 2)
            if 5 <= i and i - 5 < NDG:
                phase_d(i - 5)
            if 4 <= i and i - 4 < NDG:
                phase_c2(i - 4)
            if 3 <= i and i - 3 < NDG:
                phase_c1(i - 3)
            if 2 <= i and i - 2 < NDG:
                phase_b_post(i - 2)
            if 1 <= i and i - 1 < NDG:
                phase_b_pre(i - 1)
            if i < NDG:
                phase_a(i)
            if 1 <= i and (i - 1) % ND == 0 and (i - 1) // ND < NG:
                prologue_1((i - 1) // ND)
            if 2 <= i and (i - 2) % ND == 0 and (i - 2) // ND < NG:
                prologue_2((i - 2) // ND)
            if 3 <= i and (i - 3) % ND == 0 and (i - 3) // ND < NG:
                prologue_3((i - 3) // ND)
            if i >= ND + 4 and (i - ND - 4) % ND == 0 and (i - ND - 4) // ND < NG:
                emit_epilogue((i - ND - 4) // ND)

    nc.compile()
    return nc


_PROGRAM = None


def _get_program():
    global _PROGRAM
    if _PROGRAM is None:
        _PROGRAM = _build_program()
    return _PROGRAM


def run_on_hw(in_maps, trace=False, **kw):
    from concourse import bass_utils
    nc = _get_program()
    return bass_utils.run_bass_kernel_spmd(
        nc, in_maps, core_ids=list(range(NCORES)), trace=trace, **kw)


def make_in_maps(q_ids, c_ids, num_qs, num_cols, embed, W_h, b_h, W_o, b_o):
    W_h = np.asarray(W_h, np.float32)
    whk = np.ascontiguousarray(
        W_h.reshape(5, 128, 128).transpose(1, 0, 2).reshape(128, 5 * 128)
    ).astype(np.float16)
    w_o = np.ascontiguousarray(
        np.asarray(W_o, np.float32).reshape(D, 1)).astype(np.float16)
    b_h_col = np.ascontiguousarray(
        np.asarray(b_h, np.float32).reshape(D, 1))
    shared = dict(whk=whk, w_o=w_o, b_h=b_h_col, blockind=_BLOCKIND)
    percore = prep_all(q_ids, c_ids, num_qs, embed)
    return [dict(shared, **percore[i]) for i in range(NCORES)]


def gather_out(res, b_o):
    b_o_val = np.float32(np.asarray(b_o).reshape(-1)[0])
    outs = np.empty((B, C, 1), np.float32)
    for i in range(NCORES):
        o = np.asarray(res.results[i]["out"], np.float32)  # [pc, j]
        # pc = 64*r + col ; batch = i*BL + 2*j + r
        o = o.reshape(2, 64, PAIRS)          # [r, col, j]
        o = o.transpose(2, 0, 1).reshape(BL, C)   # [(j, r), col]
        outs[i * BL:(i + 1) * BL, :, 0] = o + b_o_val
    return outs


def kernel(q_ids, c_ids, num_qs, num_cols, embed, W_h, b_h, W_o, b_o):
    in_maps = make_in_maps(q_ids, c_ids, num_qs, num_cols, embed, W_h, b_h,
                           W_o, b_o)
    res = run_on_hw(in_maps, trace=False)
    return gather_out(res, b_o)


# revision 39
# speedup vs baseline: 1.0236x; 1.0054x over previous
"""Trainium2 Bass kernel for nn_EntityLinker (ragged_sequence).

Pure data-parallel over batch: 1024 batches -> 8 cores x 128 batches.

Gather strategy: the SWDGE fixed overhead (994ns/call) makes per-pair
indirect DMACopy gathers (576 calls/core) the bottleneck, so instead we use
the custom GPSIMD dma_gather instruction (994ns + 0.34ns/row per call,
<=896 rows/call from the descriptor-ring cap).  dma_gather needs int16
indices, so the host dedups each half-core's referenced embedding rows
(~31K distinct < 32768) into a per-half fp16 table and remaps indices to
table-local int16.  The device still gathers every reference (36MB/core)
through the DMA engines; the host only does O(refs) integer prep plus a
table layout transform.

Token sums over T=8 column tokens are fp16 identity-matmuls accumulating
in PSUM.  Attention + MLP run 2 batches per 128-partition tile, fp16
operands with fp32 PSUM accumulation, elementwise ops merged over duos
(2 pairs) to halve fixed per-op latency.  The whole per-duo dependency
chain is software-pipelined ACROSS group boundaries (engines execute
their streams in order, so a per-group pipeline would drain at each
boundary and fall behind the gather stream).  b_o is added on host.
"""

import sys

if "/opt/trn_rl_repo" not in sys.path:
    sys.path.insert(0, "/opt/trn_rl_repo")

import numpy as np

V, D = 100000, 128
B, Q, C, T = 1024, 64, 64, 8
NCORES = 8
BL = B // NCORES        # 128 batches per core
PAIRS = BL // 2         # 64 pairs (2 batches per 128-partition tile)
GP = 16                 # pairs per group
NG = PAIRS // GP        # 4 groups
ND = GP // 2            # duos per group
NDG = ND * NG           # duos per core
NB = GP + GP * T        # gather blocks per group: q then c
NH = 2                  # table halves per core (2 groups each)
TBL = 32768             # table rows per half (int16-addressable)
NEG = np.float32(-20000.0)   # fp16-safe mask bias
SCALE_SIM = float(1.0 / np.sqrt(128.0))

_P_H = np.arange(128) // 64     # which batch of the pair this partition holds
_P_C = np.arange(128) % 64      # column / q index within the batch


def _wrap16(flat):
    """dma_gather index layout: element i -> partition i%16, col i//16,
    replicated across the 8 GPSIMD sub-cores (128 partitions)."""
    w = flat.reshape(-1, 16).T          # [16, n//16]
    return np.ascontiguousarray(np.tile(w, (8, 1)))  # [128, n//16]


def _prep_core(core, q_ids, c_ids, num_qs, cnt, embed16):
    base = core * BL
    jj = np.arange(GP)

    out = dict()
    for h in range(NH):
        hb = base + h * (BL // NH)            # first batch of the half
        qh = q_ids[hb:hb + BL // NH]          # [64, Q]
        ch = c_ids[hb:hb + BL // NH]          # [64, C, T]
        refs = np.concatenate([qh.ravel(), ch.ravel()])
        uniq, inv = np.unique(refs, return_inverse=True)
        assert len(uniq) <= TBL, f"half table overflow: {len(uniq)}"
        tab = np.zeros((TBL, D), np.float16)
        tab[:len(uniq)] = embed16[uniq]
        out[f"tab{h}"] = tab
        nq = Q * (BL // NH)
        out[f"inv_q{h}"] = inv[:nq].reshape(BL // NH, Q).astype(np.int16)
        out[f"inv_c{h}"] = inv[nq:].reshape(BL // NH, C, T).astype(np.int16)

    q_idx = np.empty((NG * 128, (GP * 128) // 16), np.int16)
    c_idx = np.empty((NG * 128, (GP * T * 128) // 16), np.int16)
    qbias = np.empty((NG * 2, GP * 128), np.float16)
    qv = np.zeros((NG * 128, 2 * GP), np.float16)
    cscale = np.empty((NG * 128, GP), np.float32)

    for g in range(NG):
        h = g // 2
        inv_q = out[f"inv_q{h}"]
        inv_c = out[f"inv_c{h}"]
        # local (within-half) batch index per (partition, pair)
        lb = (g % 2) * 2 * GP + 2 * jj[None, :] + _P_H[:, None]   # [128, GP]
        cm = np.broadcast_to(_P_C[:, None], lb.shape)             # [128, GP]

        # q slots first: i = j*128 + pc
        qi = inv_q[lb, cm]                                        # [128, GP]
        # then c slots duo-major: i = u*2048 + t*256 + p2*128 + pc, so the
        # two pairs of a duo land in adjacent blocks for merged T-sums
        ci = inv_c[lb, cm]                                        # [128, GP, T]
        ci_r = ci.transpose(1, 2, 0).reshape(GP // 2, 2, T, 128)  # u, p2, t, pc
        q_idx[g * 128:(g + 1) * 128] = _wrap16(qi.T.ravel())
        c_idx[g * 128:(g + 1) * 128] = _wrap16(
            ci_r.transpose(0, 2, 1, 3).ravel())

        gbat = base + h * (BL // NH) + lb                         # global batch
        cscale[g * 128:(g + 1) * 128] = 1.0 / cnt[gbat, cm]
        for r in range(2):
            bvec = base + g * 2 * GP + 2 * jj + r
            nqs = num_qs[bvec]
            blk = np.full((GP, 128), NEG, np.float32)
            blk[:, r * 64:(r + 1) * 64] = np.where(
                np.arange(64)[None, :] < nqs[:, None], np.float32(0.0), NEG)
            qbias[g * 2 + r] = blk.reshape(-1).astype(np.float16)
            valid = (_P_C[:, None] < nqs[None, :]) & (_P_H[:, None] == r)
            qv[g * 128:(g + 1) * 128, 2 * jj + r] = \
                (valid / nqs[None, :]).astype(np.float16)

    return dict(tab0=out["tab0"], tab1=out["tab1"], q_idx=q_idx,
                c_idx=c_idx, qbias=qbias, qv=qv, cscale=cscale)


def prep_all(q_ids, c_ids, num_qs, embed):
    q_ids = np.asarray(q_ids).astype(np.int32)
    c_ids = np.asarray(c_ids).astype(np.int32)
    num_qs = np.asarray(num_qs).astype(np.int64)
    cnt = np.maximum((c_ids != 0).sum(-1), 1).astype(np.float32)     # [B, C]
    embed16 = np.asarray(embed, np.float32).astype(np.float16)
    return [_prep_core(i, q_ids, c_ids, num_qs, cnt, embed16)
            for i in range(NCORES)]


_BLOCKIND = np.zeros((2, 128), np.float16)
_BLOCKIND[0, :64] = 1.0
_BLOCKIND[1, 64:] = 1.0


def _build_program():
    from contextlib import ExitStack

    import concourse.bass as bass
    from concourse import bacc, mybir, tile
    from concourse.library_config import mlp
    from concourse.masks import make_identity

    f32 = mybir.dt.float32
    f16 = mybir.dt.float16
    i16 = mybir.dt.int16

    nc = bacc.Bacc("TRN2", target_bir_lowering=False, debug=False,
                   enable_asserts=False, num_devices=NCORES)

    tab0_d = nc.dram_tensor("tab0", [TBL, D], f16, kind="ExternalInput").ap()
    tab1_d = nc.dram_tensor("tab1", [TBL, D], f16, kind="ExternalInput").ap()
    whk_d = nc.dram_tensor("whk", [128, 5 * 128], f16, kind="ExternalInput").ap()
    w_o_d = nc.dram_tensor("w_o", [D, 1], f16, kind="ExternalInput").ap()
    b_h_d = nc.dram_tensor("b_h", [D, 1], f32, kind="ExternalInput").ap()
    blockind_d = nc.dram_tensor("blockind", [2, 128], f16, kind="ExternalInput").ap()
    q_idx_d = nc.dram_tensor("q_idx", [NG * 128, (GP * 128) // 16], i16,
                             kind="ExternalInput").ap()
    c_idx_d = nc.dram_tensor("c_idx", [NG * 128, (GP * T * 128) // 16], i16,
                             kind="ExternalInput").ap()
    qbias_d = nc.dram_tensor("qbias", [NG * 2, GP * 128], f16, kind="ExternalInput").ap()
    qv_d = nc.dram_tensor("qv", [NG * 128, 2 * GP], f16, kind="ExternalInput").ap()
    cscale_d = nc.dram_tensor("cscale", [NG * 128, GP], f32, kind="ExternalInput").ap()
    out_d = nc.dram_tensor("out", [128, PAIRS], f32, kind="ExternalOutput").ap()

    Alu = mybir.AluOpType

    with tile.TileContext(nc) as tc, ExitStack() as ctx:
        const = ctx.enter_context(tc.tile_pool(name="const", bufs=1))
        gpool = ctx.enter_context(tc.tile_pool(name="gather", bufs=3))
        spool = ctx.enter_context(tc.tile_pool(name="work", bufs=5))
        ppool = ctx.enter_context(tc.tile_pool(name="psum", bufs=5, space="PSUM"))
        tpool = ctx.enter_context(tc.tile_pool(name="psumt", bufs=2, space="PSUM"))
        gpsum = ctx.enter_context(tc.tile_pool(name="gps", bufs=1, space="PSUM"))
        hpool = ctx.enter_context(tc.tile_pool(name="hbuf", bufs=ND + 3))

        ident = const.tile([128, 128], f16)
        make_identity(nc, ident[:])
        nc.gpsimd.load_library(mlp)

        whk = const.tile([128, 5 * 128], f16)
        nc.sync.dma_start(whk[:], whk_d[:])
        w_o_t = const.tile([128, 1], f16)
        nc.sync.dma_start(w_o_t[:], w_o_d[:])
        b_h_t = const.tile([128, 1], f32)
        nc.sync.dma_start(b_h_t[:], b_h_d[:])
        blockind_t = const.tile([2, 128], f16)
        nc.sync.dma_start(blockind_t[:], blockind_d[:])
        out_sb = const.tile([128, PAIRS], f32)

        Act = mybir.ActivationFunctionType

        groups = [dict() for _ in range(NG)]
        st = [dict() for _ in range(NDG)]

        def tab_of(g):
            return tab0_d if g < NG // 2 else tab1_d

        def emit_q_io(h):
            """q gathers + mask loads for one table-half (2 groups, merged
            into one 32-block gather stream to amortize the per-call SWDGE
            overhead).  Issued >=1 group ahead of the half's compute so the
            prologue's qs matmuls never stall the in-order engine streams
            waiting on fresh gather data."""
            qidx_t = gpool.tile([128, 2 * (GP * 128) // 16], i16, tag="qidx",
                                name="qidx", bufs=NH)
            qdst = gpool.tile([128, 2 * GP, 128], f16, tag="qdst",
                              name="qdst", bufs=NH)
            for k in range(2):
                g = 2 * h + k
                G = groups[g]
                G["qbias"] = gpool.tile([2, GP * 128], f16, tag="qbias",
                                        name="qbias", bufs=NG)
                G["qv"] = gpool.tile([128, 2 * GP], f16, tag="qv", name="qv",
                                     bufs=NG)
                G["csc"] = gpool.tile([128, GP], f32, tag="csc", name="csc",
                                      bufs=NG)
                nc.sync.dma_start(
                    qidx_t[:, k * 128:(k + 1) * 128],
                    q_idx_d[g * 128:(g + 1) * 128, :])
                nc.sync.dma_start(G["qbias"][:], qbias_d[g * 2:g * 2 + 2, :])
                nc.sync.dma_start(G["qv"][:], qv_d[g * 128:(g + 1) * 128, :])
                nc.sync.dma_start(G["csc"][:],
                                  cscale_d[g * 128:(g + 1) * 128, :])
                G["qdst"] = qdst[:, k * GP:(k + 1) * GP, :]
            s = 0
            while s < 2 * GP:
                m = min(7, 2 * GP - s)
                nc.gpsimd.dma_gather(qdst[:, s:s + m, :], tab_of(2 * h)[:],
                                     qidx_t[:, s * 8:(s + m) * 8],
                                     m * 128, m * 128, D)
                s += m

        def emit_group_io(g, upto=GP * T):
            """Chunked dma_gather of the group's c tokens (duo-major).
            ``upto`` allows splitting the chunk stream so startup q gathers
            can interleave after the first few c chunks."""
            G = groups[g]
            if "cidx" not in G:
                G["cidx"] = gpool.tile([128, (GP * T * 128) // 16], i16,
                                       tag="cidx", name="cidx")
                nc.sync.dma_start(G["cidx"][:],
                                  c_idx_d[g * 128:(g + 1) * 128, :])
                G["dest"] = gpool.tile([128, GP * T, 128], f16, tag="dst",
                                       name="dest")
                G["c_done"] = 0
            dest, cidx_t = G["dest"], G["cidx"]
            s = G["c_done"]
            while s < upto:
                m = min(7, GP * T - s)
                nc.gpsimd.dma_gather(dest[:, s:s + m, :], tab_of(g)[:],
                                     cidx_t[:, s * 8:(s + m) * 8],
                                     m * 128, m * 128, D)
                s += m
            G["c_done"] = s

        # Prologue (q_summary -> per-batch MLP bias columns, transposed to
        # biasT[r, j*128+d]) is split into three stages emitted at the END
        # of successive iterations so its data wait on the fresh q gather
        # never head-of-line-blocks the per-duo pipeline.
        def prologue_1(g):
            G = groups[g]
            qdst = G["qdst"]
            qs_psA = ppool.tile([128, 512], f32, tag="psA", name="qs_psA")
            qs_ps = qs_psA[:, 0:2 * GP]
            for j in range(GP):
                nc.tensor.matmul(qs_ps[:, 2 * j:2 * j + 2],
                                 lhsT=qdst[:, j, :],
                                 rhs=G["qv"][:, 2 * j:2 * j + 2],
                                 start=True, stop=True)
            G["qs_sb"] = spool.tile([128, 2 * GP], f16, tag="qs_sb",
                                    name="qs_sb")
            nc.vector.tensor_copy(G["qs_sb"][:], qs_ps)

        def prologue_2(g):
            G = groups[g]
            bias_psA = ppool.tile([128, 512], f32, tag="psA", name="bias_psA")
            bias_ps = bias_psA[:, 0:2 * GP]
            nc.tensor.matmul(bias_ps, lhsT=whk[:, 0:128], rhs=G["qs_sb"][:],
                             start=True, stop=True)
            G["bias_sb"] = spool.tile([128, 2 * GP], f16, tag="bias_sb",
                                      name="bias_sb")
            nc.scalar.activation(G["bias_sb"][:], bias_ps, Act.Identity,
                                 bias=b_h_t[:, 0:1])

        def prologue_3(g):
            G = groups[g]
            bias_sb = G["bias_sb"]
            biasT = spool.tile([2, GP * 128], f16, tag="biasT", name="biasT")
            G["biasT"] = biasT
            for quarter in range(4):
                bt_ps = tpool.tile([128, 768], f16, tag="pst", name="bt_ps")
                for jj_ in range(4):
                    j = quarter * 4 + jj_
                    nc.tensor.transpose(bt_ps[0:2, jj_ * 128:(jj_ + 1) * 128],
                                        bias_sb[:, 2 * j:2 * j + 2], ident[:])
                nc.vector.tensor_copy(
                    biasT[:, quarter * 512:(quarter + 1) * 512],
                    bt_ps[0:2, 0:512])

        def phase_a(u):
            g, ul = u // ND, u % ND
            G, s = groups[g], st[u]
            s["psA1"] = ppool.tile([128, 512], f32, tag="psA", name="psA1")
            cs = s["psA1"][:, 0:256]
            for t in range(T):
                b = ul * 2 * T + t * 2
                nc.tensor.matmul(cs, lhsT=ident[:],
                                 rhs=G["dest"][:, b:b + 2, :],
                                 start=(t == 0), stop=(t == T - 1))
            s["c_h2"] = spool.tile([128, 256], f16, tag="c_h2", name="c_h2")
            for p in range(2):
                nc.vector.tensor_scalar_mul(
                    s["c_h2"][:, p * 128:(p + 1) * 128],
                    s["psA1"][:, p * 128:(p + 1) * 128],
                    G["csc"][:, 2 * ul + p:2 * ul + p + 1])

        def phase_b_pre(u):
            g, ul = u // ND, u % ND
            G, s = groups[g], st[u]
            s["psT"] = tpool.tile([128, 768], f16, tag="pst", name="psT")
            for p in range(2):
                nc.tensor.transpose(s["psT"][:, p * 256:p * 256 + 128],
                                    s["c_h2"][:, p * 128:(p + 1) * 128],
                                    ident[:])
                nc.tensor.transpose(s["psT"][:, p * 256 + 128:p * 256 + 256],
                                    G["qdst"][:, 2 * ul + p, :], ident[:])

        def phase_b_post(u):
            g, ul = u // ND, u % ND
            G, s = groups[g], st[u]
            s["cq_hT"] = spool.tile([128, 512], f16, tag="cq_hT", name="cq_hT")
            nc.scalar.copy(s["cq_hT"][:], s["psT"][:, 0:512])
            for p in range(2):
                j = 2 * ul + p
                sim = s["psA1"][:, 256 + p * 128:256 + (p + 1) * 128]
                nc.tensor.matmul(sim, lhsT=s["cq_hT"][:, p * 256:p * 256 + 128],
                                 rhs=s["cq_hT"][:, p * 256 + 128:p * 256 + 256],
                                 start=True, stop=False)
                nc.tensor.matmul(sim, lhsT=blockind_t[:],
                                 rhs=G["qbias"][:, j * 128:(j + 1) * 128],
                                 start=False, stop=True)
            s["att_e"] = spool.tile([128, 256], f16, tag="att_e", name="att_e")
            nc.scalar.activation(s["att_e"][:], s["psA1"][:, 256:512],
                                 Act.Exp, scale=SCALE_SIM)

        def phase_c1(u):
            g, ul = u // ND, u % ND
            G, s = groups[g], st[u]
            s_cols = spool.tile([128, 2], f32, tag="s_cols", name="s_cols")
            nc.vector.tensor_reduce(
                s_cols[:],
                s["att_e"][:].rearrange("p (x d) -> p x d", d=128),
                axis=mybir.AxisListType.X, op=Alu.add)
            r_cols = spool.tile([128, 2], f32, tag="r_cols", name="r_cols")
            nc.vector.reciprocal(r_cols[:], s_cols[:])
            att = spool.tile([128, 256], f16, tag="att", name="att")
            for p in range(2):
                nc.vector.tensor_scalar_mul(
                    att[:, p * 128:(p + 1) * 128],
                    s["att_e"][:, p * 128:(p + 1) * 128],
                    r_cols[:, p:p + 1])
                nc.tensor.transpose(
                    s["psT"][:, 512 + p * 128:512 + (p + 1) * 128],
                    att[:, p * 128:(p + 1) * 128], ident[:])
            s["attT"] = spool.tile([128, 256], f16, tag="attT", name="attT")
            nc.scalar.copy(s["attT"][:], s["psT"][:, 512:768])

        def phase_c2(u):
            g, ul = u // ND, u % ND
            G, s = groups[g], st[u]
            for p in range(2):
                nc.tensor.matmul(s["psA1"][:, p * 128:(p + 1) * 128],
                                 lhsT=G["qdst"][:, 2 * ul + p, :],
                                 rhs=s["attT"][:, p * 128:(p + 1) * 128],
                                 start=True, stop=True)
            wqT = spool.tile([128, 256], f16, tag="wqT", name="wqT")
            s["wqT"] = wqT
            nc.vector.tensor_copy(wqT[:], s["psA1"][:, 0:256])
            ch3 = spool.tile([128, 256], f16, tag="ch3", name="ch3")
            dif = spool.tile([128, 256], f16, tag="dif", name="dif")
            for p in range(2):
                c_hT = s["cq_hT"][:, p * 256:p * 256 + 128]
                wqT_p = wqT[:, p * 128:(p + 1) * 128]
                nc.vector.tensor_mul(ch3[:, p * 128:(p + 1) * 128], c_hT, wqT_p)
                nc.vector.tensor_sub(dif[:, p * 128:(p + 1) * 128], c_hT, wqT_p)
            s["ch3"] = ch3
            ch4 = spool.tile([128, 256], f16, tag="ch4", name="ch4")
            nc.scalar.activation(ch4[:], dif[:], Act.Abs)
            s["ch4"] = ch4

        def phase_d(u):
            g, ul = u // ND, u % ND
            G, s = groups[g], st[u]
            for p in range(2):
                j = 2 * ul + p
                h = s["psA1"][:, 256 + p * 128:256 + (p + 1) * 128]
                c_hT = s["cq_hT"][:, p * 256:p * 256 + 128]
                sl = slice(p * 128, (p + 1) * 128)
                for k, rhs in ((1, c_hT), (2, s["wqT"][:, sl]),
                               (3, s["ch3"][:, sl]), (4, s["ch4"][:, sl])):
                    nc.tensor.matmul(h, lhsT=whk[:, k * 128:(k + 1) * 128],
                                     rhs=rhs, start=(k == 1), stop=False)
                nc.tensor.matmul(h, lhsT=G["biasT"][:, j * 128:(j + 1) * 128],
                                 rhs=blockind_t[:], start=False, stop=True)
            s["hT"] = hpool.tile([128, 256], f16, tag="hT", name="hT")
            nc.scalar.activation(s["hT"][:], s["psA1"][:, 256:512], Act.Tanh)

        def emit_epilogue(g):
            out_ps = gpsum.tile([128, GP], f32, tag="outp", name="out_ps")
            for ul in range(ND):
                s = st[g * ND + ul]
                for p in range(2):
                    nc.tensor.matmul(
                        out_ps[:, 2 * ul + p:2 * ul + p + 1],
                        lhsT=s["hT"][:, p * 128:(p + 1) * 128],
                        rhs=w_o_t[:], start=True, stop=True)
            nc.scalar.activation(out_sb[:, g * GP:(g + 1) * GP], out_ps[:],
                                 Act.Identity)
            nc.sync.dma_start(out_d[:, g * GP:(g + 1) * GP],
                              out_sb[:, g * GP:(g + 1) * GP])

        # Global software pipeline: phases staggered across ALL duos so no
        # engine stream drains at group boundaries.  Within an iteration the
        # OLDEST phases are emitted first: engines execute their streams in
        # order, so putting the newest (gather-data-dependent) ops first
        # would head-of-line-block ready work for older duos.
        emit_q_io(0)
        for i in range(NDG + 6):
            if i < NDG and i % ND == 0:
                g = i // ND
                emit_group_io(g)
                if g == 0:
                    emit_q_io(1)
            if 5 <= i and i - 5 < NDG:
                phase_d(i - 5)
            if 4 <= i and i - 4 < NDG:
                phase_c2(i - 4)
            if 3 <= i and i - 3 < NDG:
                phase_c1(i - 3)
            if 2 <= i and i - 2 < NDG:
                phase_b_post(i - 2)
            if 1 <= i and i - 1 < NDG:
                phase_b_pre(i - 1)
            if i < NDG:
                phase_a(i)
            if 1 <= i and (i - 1) % ND == 0 and (i - 1) // ND < NG:
                prologue_1((i - 1) // ND)
            if 2 <= i and (i - 2) % ND == 0 and (i - 2) // ND < NG:
                prologue_2((i - 2) // ND)
            if 3 <= i and (i - 3) % ND == 0 and (i - 3) // ND < NG:
                prologue_3((i - 3) // ND)
            if i >= ND + 4 and (i - ND - 4) % ND == 0 and (i - ND - 4) // ND < NG:
                emit_epilogue((i - ND - 4) // ND)

    nc.compile()
    return nc


_PROGRAM = None


def _get_program():
    global _PROGRAM
    if _PROGRAM is None:
        _PROGRAM = _build_program()
    return _PROGRAM


def run_on_hw(in_maps, trace=False, **kw):
    from concourse import bass_utils
    nc = _get_program()
    return bass_utils.run_bass_kernel_spmd(
        nc, in_maps, core_ids=list(range(NCORES)), trace=trace, **kw)


def make_in_maps(q_ids, c_ids, num_qs, num_cols, embed, W_h, b_h, W_o, b_o):
    W_h = np.asarray(W_h, np.float32)
    whk = np.ascontiguousarray(
        W_h.reshape(5, 128, 128).transpose(1, 0, 2).reshape(128, 5 * 128)
    ).astype(np.float16)
    w_o = np.ascontiguousarray(
        np.asarray(W_o, np.float32).reshape(D, 1)).astype(np.float16)
    b_h_col = np.ascontiguousarray(
        np.asarray(b_h, np.float32).reshape(D, 1))
    shared = dict(whk=whk, w_o=w_o, b_h=b_h_col, blockind=_BLOCKIND)
    percore = prep_all(q_ids, c_ids, num_qs, embed)
    return [dict(shared, **percore[i]) for i in range(NCORES)]


def gather_out(res, b_o):
    b_o_val = np.float32(np.asarray(b_o).reshape(-1)[0])
    outs = np.empty((B, C, 1), np.float32)
    for i in range(NCORES):
        o = np.asarray(res.results[i]["out"], np.float32)  # [pc, j]
        # pc = 64*r + col ; batch = i*BL + 2*j + r
        o = o.reshape(2, 64, PAIRS)          # [r, col, j]
        o = o.transpose(2, 0, 1).reshape(BL, C)   # [(j, r), col]
        outs[i * BL:(i + 1) * BL, :, 0] = o + b_o_val
    return outs


def kernel(q_ids, c_ids, num_qs, num_cols, embed, W_h, b_h, W_o, b_o):
    in_maps = make_in_maps(q_ids, c_ids, num_qs, num_cols, embed, W_h, b_h,
                           W_o, b_o)
    res = run_on_hw(in_maps, trace=False)
    return gather_out(res, b_o)
